# revision 53
# baseline (speedup 1.0000x reference)
"""Trainium2 Bass kernel for a classic Mamba block (B=2, L=2048, Dm=1024,
E=2048, N=16, R=64, K=3) running SPMD on 8 NeuronCores.

Sharding: tensor-parallel on inner dim E (E_loc = 256 per core).

v2 architecture ("layout D"): the selective scan keeps 128 e-channels in
SBUF partitions and time in the free dim; the N=16 ssm states are processed
as 16 sequential scan tiles per (batch, e-subtile).  delta/du are consumed
directly from SBUF (no DRAM broadcast round-trip, which gated v1); only the
small [16, L] B/C rows are partition-broadcast.  The n-contraction
(y = sum_n C_n * h_n) is identity-matmul PSUM accumulation on TensorE, the
D*u skip term is a diag(D) matmul into the same PSUM bank, the causal
depthwise conv is 3 diag(w) matmuls, and silu(z) gating is fused into the
PSUM drain.  Collectives: one merged AllReduce per batch for the selective
projection; the output-projection ReduceScatter runs as quarter chunks for
batch 0 (overlapped with batch 1's scan) and one full-batch RS for batch 1
(RS latency is sublinear in size, so one big call shortens the exposed
tail).
"""

import sys

if "/opt/trn_rl_repo" not in sys.path:
    sys.path.insert(0, "/opt/trn_rl_repo")

import numpy as np

# ---------------------------------------------------------------------------
# Problem constants (hardcoded per contract)
B = 2
L = 2048          # sequence length per batch
DM = 1024         # model dim
E = 2048          # inner dim
N = 16            # ssm state dim
R = 64            # dt rank
K = 3             # conv kernel
N_CORES = 8
E_LOC = E // N_CORES          # 256
NS = E_LOC // 128             # e-subtiles per core (2)

FC = 512                      # psum free chunk (one bank)

# knobs (GpSimd offload measured counterproductive: Pool shares the SBUF
# port with DVE and slows the scans ~50%)
GPS_HC = ()                   # n-indices whose hc-mul runs on GpSimd
GPS_BM = ()                   # n-indices whose b-mul runs on GpSimd

_PROGRAM_CACHE = {}


def build_program(Lb=L):
    key = (Lb, tuple(GPS_HC), tuple(GPS_BM))
    if key in _PROGRAM_CACHE:
        return _PROGRAM_CACHE[key]

    import concourse.bacc as bacc
    import concourse.mybir as mybir
    import concourse.tile as tile
    import concourse.tile_utils as tile_utils
    import concourse.bass as _bass

    if getattr(tile_utils, "max_sbuf_usage", None) is not None:
        tile_utils.max_sbuf_usage = max(tile_utils.max_sbuf_usage, 207 * 1024)

    f32 = mybir.dt.float32
    bf16 = mybir.dt.bfloat16
    f16 = mybir.dt.float16
    AF = mybir.ActivationFunctionType
    OP = mybir.AluOpType

    tok = B * Lb
    n_fc = Lb // FC               # 4 psum chunks per full-L tile
    FH = Lb // 2                  # in-proj token half
    QT = Lb // 4                  # RS quarter (tokens)
    HR = QT // N_CORES            # rows per rank per quarter

    nc = bacc.Bacc("TRN2", target_bir_lowering=False, debug=False,
                   num_devices=N_CORES)

    # ---------------- DRAM I/O ----------------
    xT = nc.dram_tensor("xT", [DM, tok], f16, kind="ExternalInput")
    w_inT = nc.dram_tensor("w_inT", [DM, 2 * E_LOC], f16, kind="ExternalInput")
    conv_diag = nc.dram_tensor("conv_diag", [128, NS * K * 128], f16,
                               kind="ExternalInput")
    conv_b = nc.dram_tensor("conv_b", [128, NS], f32, kind="ExternalInput")
    w_selT = nc.dram_tensor("w_selT", [128, NS * (R + 2 * N)], f16,
                            kind="ExternalInput")
    dt_wT = nc.dram_tensor("dt_wT", [R, E_LOC], f16, kind="ExternalInput")
    dt_b = nc.dram_tensor("dt_b", [128, NS], f32, kind="ExternalInput")
    a_cols = nc.dram_tensor("a_cols", [128, NS * N], f32, kind="ExternalInput")
    ident = nc.dram_tensor("ident", [128, 128], bf16, kind="ExternalInput")
    d_diag = nc.dram_tensor("d_diag", [128, NS * 128], f16,
                            kind="ExternalInput")
    w_outT = nc.dram_tensor("w_outT", [128, NS * DM], f16,
                            kind="ExternalInput")

    out_loc = nc.dram_tensor("out_loc", [tok // N_CORES, DM], bf16,
                             kind="ExternalOutput")

    # internal DRAM
    ar_in = [nc.dram_tensor(f"ar_in{b}", [R + 2 * N, Lb], f32)
             for b in range(B)]
    ar_out = [nc.dram_tensor(f"ar_out{b}", [R + 2 * N, Lb], f32,
                             addr_space="Shared") for b in range(B)]
    bc_sp = [nc.dram_tensor(f"bc_sp{b}", [2 * N, Lb], bf16) for b in range(B)]
    part = [nc.dram_tensor(f"part{b}", [Lb, DM], bf16) for b in range(B)]
    rs_out = [[nc.dram_tensor(f"rs_out{b}_{q}", [HR, DM], bf16)
               for q in range(4)] for b in range(B)]
    rs_out1 = nc.dram_tensor("rs_out1", [Lb // N_CORES, DM], bf16)

    rg = [list(range(N_CORES))]

    def bcast_row(dram_t, row, lo, hi):
        """AP reading DRAM row `row` cols [lo:hi) broadcast to 128
        partitions."""
        sl = dram_t[row:row + 1, lo:hi]
        return _bass.AP(tensor=sl.tensor, offset=sl.offset,
                        ap=[[0, 128], list(sl.ap[1])])

    with tile.TileContext(nc) as tc:
        with tc.tile_pool(name="consts", bufs=1) as consts, \
             tc.tile_pool(name="pbig", bufs=1, space="PSUM") as pbig, \
             tc.tile_pool(name="pchunk", bufs=4, space="PSUM") as pchunk, \
             tc.tile_pool(name="xt", bufs=9) as xt_pool, \
             tc.tile_pool(name="xc", bufs=2) as xc_pool, \
             tc.tile_pool(name="u", bufs=4) as u_pool, \
             tc.tile_pool(name="z", bufs=4) as z_pool, \
             tc.tile_pool(name="small", bufs=2) as small_pool, \
             tc.tile_pool(name="stage", bufs=1) as stage_pool, \
             tc.tile_pool(name="dbcp", bufs=1) as dbc_pool, \
             tc.tile_pool(name="dd", bufs=2) as dd_pool, \
             tc.tile_pool(name="rep", bufs=6) as rep_pool, \
             tc.tile_pool(name="sw", bufs=6) as sw_pool, \
             tc.tile_pool(name="y", bufs=3) as y_pool, \
             tc.tile_pool(name="gz", bufs=2) as gz_pool, \
             tc.tile_pool(name="st", bufs=5) as st_pool:

            # ---- constants ----
            w_inT_sb = consts.tile([128, DM // 128, 2 * E_LOC], f16)
            nc.sync.dma_start(out=w_inT_sb[:], in_=w_inT[:].rearrange(
                "(k p) m -> p k m", p=128))
            conv_diag_sb = consts.tile([128, NS, K, 128], f16)
            nc.sync.dma_start(out=conv_diag_sb[:], in_=conv_diag[:].rearrange(
                "p (s k m) -> p s k m", s=NS, k=K))
            conv_b_sb = consts.tile([128, NS], f32)
            nc.sync.dma_start(out=conv_b_sb[:], in_=conv_b[:])
            w_selT_sb = consts.tile([128, NS, R + 2 * N], f16)
            nc.sync.dma_start(out=w_selT_sb[:], in_=w_selT[:].rearrange(
                "p (s m) -> p s m", s=NS))
            dt_wT_sb = consts.tile([R, E_LOC], f16)
            nc.sync.dma_start(out=dt_wT_sb[:], in_=dt_wT[:])
            dt_b_sb = consts.tile([128, NS], f32)
            nc.sync.dma_start(out=dt_b_sb[:], in_=dt_b[:])
            a_cols_sb = consts.tile([128, NS * N], f32)
            nc.sync.dma_start(out=a_cols_sb[:], in_=a_cols[:])
            ident_sb = consts.tile([128, 128], bf16)
            nc.sync.dma_start(out=ident_sb[:], in_=ident[:])
            d_diag_sb = consts.tile([128, NS, 128], f16)
            nc.sync.dma_start(out=d_diag_sb[:], in_=d_diag[:].rearrange(
                "p (s m) -> p s m", s=NS))
            w_outT_sb = consts.tile([128, NS, DM], f16)
            nc.sync.dma_start(out=w_outT_sb[:], in_=w_outT[:].rearrange(
                "p (s m) -> p s m", s=NS))

            u_tiles = {}
            z_tiles = {}
            y_tiles = {}

            # ================= phase 1 (per batch) =================
            def phase1(b):
                """in-proj, conv, dbc, AllReduce, z for batch b (generator:
                yields between emission chunks for interleaving).  The xc
                half -> conv -> dbc path runs per token-half so the
                AllReduce is dispatched as early as possible; the z halves
                (m=2,3) are emitted after it."""
                xc_tiles = {s: xc_pool.tile([128, Lb], f16, tag="xc",
                                            name=f"xc_{b}_{s}")
                            for s in range(NS)}
                for s in range(NS):
                    u_tiles[(b, s)] = u_pool.tile([128, Lb], f16, tag="u",
                                                  name=f"u_{b}_{s}")
                    z_tiles[(b, s)] = z_pool.tile([128, Lb], bf16, tag="z",
                                                  name=f"z_{b}_{s}")
                dbc_sb = dbc_pool.tile([R + 2 * N, Lb], f32, tag="dbc",
                                       name=f"dbc_{b}")
                xt_tiles = {}

                def load_xt(fh):
                    xt_tiles[fh] = []
                    for k in range(DM // 128):
                        t = xt_pool.tile([128, FH], f16, tag="xt")
                        nc.sync.dma_start(
                            out=t[:],
                            in_=xT[k * 128:(k + 1) * 128,
                                   b * Lb + fh * FH:b * Lb + (fh + 1) * FH])
                        xt_tiles[fh].append(t)

                def mgroup(fh, m):
                    s = m % 2
                    for c in range(FH // FC):
                        pc = pchunk.tile([128, FC], f32, tag="pc",
                                         name=f"pin_{b}_{fh}_{m}_{c}")
                        for k in range(DM // 128):
                            nc.tensor.matmul(
                                pc[:],
                                lhsT=w_inT_sb[:, k, m * 128:(m + 1) * 128],
                                rhs=xt_tiles[fh][k][:, c * FC:(c + 1) * FC],
                                start=(k == 0), stop=(k == DM // 128 - 1))
                        off = fh * FH + c * FC
                        if m < 2:
                            nc.scalar.copy(xc_tiles[s][:, off:off + FC], pc[:])
                        else:
                            nc.scalar.copy(
                                z_tiles[(b, s)][:, off:off + FC], pc[:])

                def conv_chunk(s, c):
                    xc = xc_tiles[s]
                    lo = c * FC
                    pcv = pchunk.tile([128, FC], f32, tag="pc",
                                      name=f"pcv_{b}_{s}_{c}")
                    nc.tensor.matmul(
                        pcv[:], lhsT=conv_diag_sb[:, s, 2, :],
                        rhs=xc[:, lo:lo + FC], start=True, stop=False)
                    e1 = 1 if c == 0 else 0
                    nc.tensor.matmul(
                        pcv[:, e1:FC], lhsT=conv_diag_sb[:, s, 1, :],
                        rhs=xc[:, lo + e1 - 1:lo + FC - 1],
                        start=False, stop=False)
                    e2 = 2 if c == 0 else 0
                    nc.tensor.matmul(
                        pcv[:, e2:FC], lhsT=conv_diag_sb[:, s, 0, :],
                        rhs=xc[:, lo + e2 - 2:lo + FC - 2],
                        start=False, stop=True)
                    sg = st_pool.tile([128, FC], f32, tag="st",
                                      name=f"sgc_{b}_{s}_{c}")
                    nc.scalar.activation(sg[:], pcv[:], AF.Sigmoid,
                                         bias=conv_b_sb[:, s:s + 1])
                    nc.vector.scalar_tensor_tensor(
                        u_tiles[(b, s)][:, lo:lo + FC], pcv[:],
                        conv_b_sb[:, s:s + 1], sg[:],
                        op0=OP.add, op1=OP.mult)

                def dbc_chunk(c):
                    pd = pchunk.tile([R + 2 * N, FC], f32, tag="pc",
                                     name=f"pdbc_{b}_{c}")
                    for s in range(NS):
                        nc.tensor.matmul(
                            pd[:], lhsT=w_selT_sb[:, s, :],
                            rhs=u_tiles[(b, s)][:, c * FC:(c + 1) * FC],
                            start=(s == 0), stop=(s == NS - 1))
                    cs = slice(c * FC, (c + 1) * FC)
                    nc.scalar.copy(dbc_sb[:, cs], pd[:])
                    nc.sync.dma_start(out=ar_in[b][:, cs], in_=dbc_sb[:, cs])

                load_xt(0)
                yield
                for fh in range(2):
                    for m in range(2):
                        mgroup(fh, m)
                        yield
                    if fh == 0:
                        load_xt(1)
                    for s in range(NS):
                        for c in range(fh * 2, fh * 2 + 2):
                            conv_chunk(s, c)
                        yield
                    for c in range(fh * 2, fh * 2 + 2):
                        dbc_chunk(c)
                    yield
                nc.gpsimd.collective_compute(
                    "AllReduce", OP.add, replica_groups=rg,
                    ins=[ar_in[b][:]], outs=[ar_out[b][:]])
                yield
                for fh in range(2):
                    load_xt(fh)          # reload (xt pool too small for both)
                    yield
                    for m in range(2, 4):
                        mgroup(fh, m)
                        yield

            # ================= phase 2 =================
            def prep_batch(b):
                """Stage AllReduce output; dtlow fp16 + B/C rows to DRAM."""
                stage = stage_pool.tile([R + 2 * N, Lb], f32, tag="stage",
                                        name=f"stage_{b}")
                for i in range(4):
                    cs = slice(i * FC, (i + 1) * FC)
                    nc.sync.dma_start(out=stage[:, cs], in_=ar_out[b][:, cs])
                dtlow = dd_pool.tile([R, Lb], f16, tag="dtlow",
                                     name=f"dtlow_{b}")
                nc.scalar.copy(dtlow[:], stage[0:R, :])
                btct = small_pool.tile([2 * N, Lb], bf16, tag="btct",
                                       name=f"btct_{b}")
                nc.vector.tensor_copy(btct[:], stage[R:R + 2 * N, :])
                for i in range(2):
                    cs = slice(i * FH, (i + 1) * FH)
                    nc.sync.dma_start(out=bc_sp[b][:, cs], in_=btct[:, cs])
                return dtlow

            def prep_s(b, s, dtlow):
                """delta (softplus) and du for (b, s)."""
                delta = dd_pool.tile([128, Lb], f16, tag="delta",
                                     name=f"delta_{b}_{s}")
                ets = []
                for c in range(n_fc):
                    pd = pchunk.tile([128, FC], f32, tag="pc",
                                     name=f"pdt_{b}_{s}_{c}")
                    nc.tensor.matmul(
                        pd[:], lhsT=dt_wT_sb[:, s * 128:(s + 1) * 128],
                        rhs=dtlow[:, c * FC:(c + 1) * FC],
                        start=True, stop=True)
                    et = st_pool.tile([128, FC], f32, tag="st",
                                      name=f"et_{b}_{s}_{c}")
                    nc.scalar.activation(et[:], pd[:], AF.Exp,
                                         bias=dt_b_sb[:, s:s + 1])
                    ets.append(et)
                for c in range(n_fc):
                    nc.scalar.activation(delta[:, c * FC:(c + 1) * FC],
                                         ets[c], AF.Ln, bias=1.0)
                du = dd_pool.tile([128, Lb], f16, tag="du",
                                  name=f"du_{b}_{s}")
                nc.vector.tensor_mul(du[:], delta[:], u_tiles[(b, s)][:])
                return delta, du

            def scan_s(b, s, delta, du, bg=None, bg_steps=1, segs=None,
                       seg_hook=None):
                """16-state scan for (b, s); returns the open PSUM y tile.
                bg: generator to step between n-iterations (interleave).
                segs: optional chained token segments [(lo, hi), ...];
                seg_hook(si, lo, hi, py) runs after a segment's y (incl.
                skip term) is accumulated."""
                if segs is None:
                    segs = [(0, Lb)]
                py = pbig.tile([128, Lb], f32, tag="pbig", name=f"py_{b}_{s}")
                carry = None
                if len(segs) > 1:
                    carry = dd_pool.tile([128, N], f32, tag="carry",
                                         name=f"carry_{b}_{s}")
                reps = {}
                order = [(si, n, lo, hi) for si, (lo, hi) in enumerate(segs)
                         for n in range(N)]

                def fetch(i):
                    if i >= len(order):
                        return
                    si, n, lo, hi = order[i]
                    w = hi - lo
                    br = rep_pool.tile([128, Lb], bf16, tag="rep",
                                       name=f"br_{b}_{s}_{lo}_{n}")
                    nc.sync.dma_start(out=br[:, 0:w],
                                      in_=bcast_row(bc_sp[b], n, lo, hi))
                    cr = rep_pool.tile([128, Lb], bf16, tag="rep",
                                       name=f"cr_{b}_{s}_{lo}_{n}")
                    nc.sync.dma_start(out=cr[:, 0:w],
                                      in_=bcast_row(bc_sp[b], N + n, lo, hi))
                    reps[(si, n)] = (br, cr)

                fetch(0)
                fetch(1)
                for si, (lo, hi) in enumerate(segs):
                    w = hi - lo
                    for n in range(N):
                        fetch(si * N + n + 2)
                        br, cr = reps.pop((si, n))
                        a_sb = sw_pool.tile([128, Lb], bf16, tag="sw",
                                            name=f"a_{b}_{s}_{lo}_{n}")
                        nc.scalar.activation(
                            a_sb[:, 0:w], delta[:, lo:hi], AF.Exp,
                            scale=a_cols_sb[:, s * N + n:s * N + n + 1])
                        b_sb = sw_pool.tile([128, Lb], bf16, tag="sw",
                                            name=f"b_{b}_{s}_{lo}_{n}")
                        nc.vector.tensor_mul(b_sb[:, 0:w], du[:, lo:hi],
                                             br[:, 0:w])
                        h_sb = sw_pool.tile([128, Lb], bf16, tag="sw",
                                            name=f"h_{b}_{s}_{lo}_{n}")
                        init = 0.0 if si == 0 else carry[:, n:n + 1]
                        nc.vector.tensor_tensor_scan(
                            h_sb[:, 0:w], a_sb[:, 0:w], b_sb[:, 0:w], init,
                            op0=OP.mult, op1=OP.add)
                        if si + 1 < len(segs):
                            nc.vector.tensor_copy(carry[:, n:n + 1],
                                                  h_sb[:, w - 1:w])
                        hc_sb = sw_pool.tile([128, Lb], bf16, tag="sw",
                                             name=f"hc_{b}_{s}_{lo}_{n}")
                        nc.vector.tensor_mul(hc_sb[:, 0:w], h_sb[:, 0:w],
                                             cr[:, 0:w])
                        for c in range(lo // FC, hi // FC):
                            nc.tensor.matmul(
                                py[:, c * FC:(c + 1) * FC], lhsT=ident_sb[:],
                                rhs=hc_sb[:, c * FC - lo:(c + 1) * FC - lo],
                                start=(n == 0), stop=False)
                        if bg is not None:
                            for _ in range(bg_steps):
                                next(bg, None)
                    # skip term: py += diag(D) @ u for this segment
                    for c in range(lo // FC, hi // FC):
                        nc.tensor.matmul(
                            py[:, c * FC:(c + 1) * FC],
                            lhsT=d_diag_sb[:, s, :],
                            rhs=u_tiles[(b, s)][:, c * FC:(c + 1) * FC],
                            start=False, stop=True)
                    if seg_hook is not None:
                        seg_hook(si, lo, hi, py)
                return py

            ysil = {}

            def yasm(b, s, py, lo=0, hi=None):
                """Drain PSUM y [lo:hi) through silu(z) gating into fp16."""
                hi = Lb if hi is None else hi
                z = z_tiles[(b, s)]
                if (b, s) not in y_tiles:
                    sg = gz_pool.tile([128, Lb], bf16, tag="sg",
                                      name=f"sgz_{b}_{s}")
                    nc.scalar.activation(sg[:], z[:], AF.Sigmoid)
                    yg1 = gz_pool.tile([128, Lb], bf16, tag="yg1",
                                       name=f"yg1_{b}_{s}")
                    y_tiles[(b, s)] = y_pool.tile([128, Lb], f16, tag="y",
                                                  name=f"yg_{b}_{s}")
                    ysil[(b, s)] = (sg, yg1)
                sg, yg1 = ysil[(b, s)]
                yg = y_tiles[(b, s)]
                # drain PSUM via Act so both gating muls run in 2x DVE mode
                nc.scalar.copy(yg1[:, lo:hi], py[:, lo:hi])
                tmp = sw_pool.tile([128, Lb], bf16, tag="sw",
                                   name=f"ytmp_{b}_{s}_{lo}")
                nc.vector.tensor_mul(tmp[:, 0:hi - lo], yg1[:, lo:hi],
                                     z[:, lo:hi])
                nc.vector.tensor_mul(yg[:, lo:hi], tmp[:, 0:hi - lo],
                                     sg[:, lo:hi])

            def outproj_mt(b, mt):
                """One 128-token out-proj block: po chunks -> part DRAM."""
                for f in range(DM // FC):
                    po = pchunk.tile([128, FC], f32, tag="pc",
                                     name=f"po_{b}_{mt}_{f}")
                    for s in range(NS):
                        nc.tensor.matmul(
                            po[:],
                            lhsT=y_tiles[(b, s)][:, mt * 128:(mt + 1) * 128],
                            rhs=w_outT_sb[:, s, f * FC:(f + 1) * FC],
                            start=(s == 0), stop=(s == NS - 1))
                    sto = st_pool.tile([128, FC], bf16, tag="st",
                                       name=f"sto_{b}_{mt}_{f}")
                    nc.scalar.copy(sto[:], po[:])
                    nc.sync.dma_start(
                        out=part[b][mt * 128:(mt + 1) * 128,
                                    f * FC:(f + 1) * FC],
                        in_=sto[:])

            def outproj0():
                """Batch-0 out-proj with quarter ReduceScatters
                (generator)."""
                for mt in range(Lb // 128):
                    outproj_mt(0, mt)
                    yield
                    if mt % 4 == 3:
                        q = mt // 4
                        nc.gpsimd.collective_compute(
                            "ReduceScatter", OP.add, replica_groups=rg,
                            ins=[part[0][q * QT:(q + 1) * QT, :]],
                            outs=[rs_out[0][q][:]])
                        nc.sync.dma_start(
                            out=out_loc[q * HR:(q + 1) * HR, :],
                            in_=rs_out[0][q][:])
                        yield

            def rs_b1():
                # tail batch: one big RS beats 4 serial quarter-RS
                nc.gpsimd.collective_compute(
                    "ReduceScatter", OP.add, replica_groups=rg,
                    ins=[part[1][:]], outs=[rs_out1[:]])
                nc.sync.dma_start(out=out_loc[4 * HR:, :], in_=rs_out1[:])

            def run_gen(g):
                for _ in g:
                    pass

            # --------- emission schedule ---------
            def chain(*gens):
                for g in gens:
                    yield from g

            g_p1_0 = phase1(0)
            for _ in range(12):                  # through AR(b0) dispatch
                next(g_p1_0, None)
            g_p1_1 = phase1(1)
            for _ in range(12):                  # through AR(b1) dispatch
                next(g_p1_1, None)
            g_rest = chain(g_p1_0, g_p1_1)       # z-groups of both batches
            dtlow0 = prep_batch(0)
            d0, du0 = prep_s(0, 0, dtlow0)
            py = scan_s(0, 0, d0, du0, bg=g_rest)
            yasm(0, 0, py)
            d1, du1 = prep_s(0, 1, dtlow0)
            dtlow1 = prep_batch(1)       # batch-1 prep under scan(0,1)
            d2, du2 = prep_s(1, 0, dtlow1)
            py = scan_s(0, 1, d1, du1, bg=g_rest)
            run_gen(g_rest)
            yasm(0, 1, py)
            g_op0 = outproj0()
            py = scan_s(1, 0, d2, du2, bg=g_op0)
            yasm(1, 0, py)
            d3, du3 = prep_s(1, 1, dtlow1)
            py = scan_s(1, 1, d3, du3, bg=g_op0)
            run_gen(g_op0)
            yasm(1, 1, py)
            for mt in range(Lb // 128):
                outproj_mt(1, mt)
            rs_b1()

    nc.compile()
    _PROGRAM_CACHE[key] = nc
    return nc


# ---------------------------------------------------------------------------
def host_prep(inputs, Lb=L):
    x = np.asarray(inputs["x"], np.float32)
    W_in = np.asarray(inputs["W_in"], np.float32)
    conv_w = np.asarray(inputs["conv_w"], np.float32)
    conv_b = np.asarray(inputs["conv_b"], np.float32)
    W_sel = np.asarray(inputs["W_sel"], np.float32)
    dt_w = np.asarray(inputs["dt_w"], np.float32)
    dt_b = np.asarray(inputs["dt_b"], np.float32)
    A_log = np.asarray(inputs["A_log"], np.float32)
    D_param = np.asarray(inputs["D_param"], np.float32)
    W_out = np.asarray(inputs["W_out"], np.float32)

    import ml_dtypes
    bf16 = ml_dtypes.bfloat16
    tok = B * Lb
    xT = np.ascontiguousarray(
        x[:, :Lb, :].reshape(tok, DM).T).astype(np.float16)
    A = -np.exp(A_log.astype(np.float64)).astype(np.float32)   # [E, N]

    ident = np.eye(128, dtype=np.float32)

    in_maps = []
    for k in range(N_CORES):
        es = slice(k * E_LOC, (k + 1) * E_LOC)
        W_in_loc = np.concatenate([W_in[k * E_LOC:(k + 1) * E_LOC],
                                   W_in[E + k * E_LOC:E + (k + 1) * E_LOC]],
                                  axis=0)            # [2*E_LOC, DM]
        A_loc = A[es]                                # [E_LOC, N]

        # a_cols[p, s*N + n] = A_loc[s*128+p, n]
        a_cols = np.zeros((128, NS * N), np.float32)
        for s in range(NS):
            for n in range(N):
                a_cols[:, s * N + n] = A_loc[s * 128:(s + 1) * 128, n]

        # conv_diag[p, s, kk, :] = diag of conv_w[es][s*128+p] tap kk
        conv_diag = np.zeros((128, NS, K, 128), np.float32)
        for s in range(NS):
            for kk in range(K):
                conv_diag[:, s, kk, :] = np.diag(
                    conv_w[es][s * 128:(s + 1) * 128, 0, kk])
        d_diag = np.zeros((128, NS, 128), np.float32)
        for s in range(NS):
            d_diag[:, s, :] = np.diag(D_param[es][s * 128:(s + 1) * 128])

        def two(v):  # [E_LOC] -> [128, NS]
            return np.ascontiguousarray(v.reshape(NS, 128).T)

        in_maps.append({
            "xT": xT,
            "w_inT": np.ascontiguousarray(W_in_loc.T).astype(np.float16),
            "conv_diag": np.ascontiguousarray(
                conv_diag.reshape(128, NS * K * 128)).astype(np.float16),
            "conv_b": two(conv_b[es]),
            "w_selT": np.ascontiguousarray(
                W_sel[:, es].T.reshape(NS, 128, R + 2 * N).transpose(
                    1, 0, 2).reshape(128, NS * (R + 2 * N))).astype(
                        np.float16),
            "dt_wT": np.ascontiguousarray(dt_w[es].T).astype(np.float16),
            "dt_b": two(dt_b[es]),
            "a_cols": a_cols,
            "ident": ident.astype(bf16),
            "d_diag": np.ascontiguousarray(
                d_diag.reshape(128, NS * 128)).astype(np.float16),
            "w_outT": np.ascontiguousarray(
                W_out[:, es].T.reshape(NS, 128, DM).transpose(
                    1, 0, 2).reshape(128, NS * DM)).astype(np.float16),
        })
    return in_maps


def assemble_output(results, Lb=L):
    out = np.empty((B, Lb, DM), np.float32)
    QT = Lb // 4
    hr = QT // N_CORES
    lr = Lb // N_CORES
    for c in range(N_CORES):
        chunk = np.asarray(results[c]["out_loc"], np.float32)
        for q in range(4):       # batch 0: quarter ReduceScatters
            out[0, q * QT + c * hr:q * QT + (c + 1) * hr, :] = \
                chunk[q * hr:(q + 1) * hr, :]
        # batch 1: one full-batch ReduceScatter
        out[1, c * lr:(c + 1) * lr, :] = chunk[4 * hr:4 * hr + lr, :]
    return out


def kernel(**inputs) -> np.ndarray:
    from concourse import bass_utils
    nc = build_program()
    in_maps = host_prep(inputs)
    res = bass_utils.run_bass_kernel_spmd(nc, in_maps, list(range(N_CORES)))
    return assemble_output(res.results).astype(np.float32)
